# revision 16
# baseline (speedup 1.0000x reference)
"""Trainium2 Bass kernel for nn_CatEncoderCross.

Computes out[b,i,j,:] = input1[b,i,:] @ W[:768] + input2[b,j,:] @ W[768:] + bias
for shapes input1/input2 [4,128,768], W [1536,768], b [768],
output [4,128,128,768] f32 (~201 MB).

Sharding (batch, dout-half): core c handles batch c//2 and output columns
[384*(c%2), 384*(c%2)+384).

Per core, with j on partitions:
  p2  = x2 @ W2h              [128(j), 384]   (PE, f32 psum -> bf16 SBUF)
  p1T = (x1 @ W1h + bias)^T   [384(d), 128(i)] computed directly in the
        transposed layout (lhsT = W1 blocks, rhs = x1T chunks), bf16 SBUF.
  rows are produced in 4-row tiles.  The p1 broadcast uses PE *transpose*
  matmuls: in_ = p1T[:, chunk, i] with free-dim stride 0 and identity as
  the permutation operand transposes a 128-wide broadcast of p1[i] into a
  BF16 psum tile (TRN2 transpose may write 16-bit psum).  Keeping psum in
  bf16 makes every consumer eligible for cheap 16-bit modes:
    D tiles: DVE tensor_add (2x mode: all operands 16-bit packed)
             psum + p2 bcast -> ob f16
    P tiles: Act copies psum -> bf16 staging viewing pairs as f32 (pure
             byte move, halves billed elements), Pool (gpsimd) adds p2
  all rows land in one resident SBUF tile ob [128(j), 128(i), 384(d)] f16.

Output egress uses the KVWritebackAnt SWDGE instruction instead of bulk
DMA: with ctx_idx=0, batch=i-rows, d_head=128 (the j partitions), and the
row of 384 f16 viewed as ncn=192 f32, each call writes
out[i, j, :] = ob[j, i, :] for a chunk of i-rows directly in the required
[i, j, d] HBM layout.

Output is written fp16 (host upcasts); inputs are bf16.  Rel err ~4e-3 vs
the f32 reference (gate 2e-2).
"""

import os
import numpy as np

P = 128
DO = 384  # output columns per core (dout/2)
DOH = DO // 2  # f32-pair view of a row
DC = 3  # d-chunks of 128 in DO
KO = 6  # K chunks of 128 in d1 (=d2)
NI = 128  # n1 rows per core (full batch)
NJ = 128  # n2
NCORES = 8
FUSE = 8  # rows per psum tile

PSB_BUFS = int(os.environ.get("KERNEL_PSB_BUFS", "2"))
PC_BUFS = int(os.environ.get("KERNEL_PC_BUFS", "2"))
WARM_MMS = int(os.environ.get("KERNEL_WARM_MMS", "36"))
# per-16-tile path pattern (8 rows per tile): D=DVE 2x add,
# P=Act bitcast copy + Pool add.  P tiles sit early/mid so the Pool engine
# is free for the trailing kv_writeback desc-gens.
PATTERN = os.environ.get("KERNEL_PATTERN", "DDPDDDPDDDPDDDDD")
# i-row chunk sizes per kv_writeback call
WB_CHUNKS = tuple(
    int(x) for x in os.environ.get("KERNEL_WB", "40,40,32,16").split(",")
)
# perf-probe only: comma list of stages to skip (never set in real runs)
SKIP = set(s for s in os.environ.get("KERNEL_SKIP", "").split(",") if s)

_cache = {}


def _build_module():
    import concourse.bacc as bacc
    import concourse.mybir as mybir
    import concourse.tile as tile

    F32 = mybir.dt.float32
    F16 = mybir.dt.float16
    I32 = mybir.dt.int32
    BF16 = mybir.dt.bfloat16

    assert sum(WB_CHUNKS) == NI
    assert len(PATTERN) == NI // FUSE and set(PATTERN) <= set("DP")

    nc = bacc.Bacc("TRN2", target_bir_lowering=False, debug=False)

    # packed inputs: inA = [x1T(768) | eye(128) | w1(2304)] cols,
    #                inB = [x2T(768) | w2(2304)] cols — few big DMAs beat
    # many small ones (the SP sequencer issues one DMA per ~650 ns)
    CA = KO * P + P + KO * DO
    CB = KO * P + KO * DO
    inA_d = nc.dram_tensor("inA", [P, CA], BF16, kind="ExternalInput")
    inB_d = nc.dram_tensor("inB", [P, CB], BF16, kind="ExternalInput")
    bias_d = nc.dram_tensor("biasr", [1, DO], BF16, kind="ExternalInput")
    # out rows hold 384 f16 viewed as 192 f32 (host bitcasts back)
    out_d = nc.dram_tensor("out", [NI, NJ, DOH], F32, kind="ExternalOutput")

    with tile.TileContext(nc) as tc:
        with (
            tc.tile_pool(name="const", bufs=1) as cpool,
            tc.tile_pool(name="psb", bufs=PSB_BUFS, space="PSUM") as psbpool,
            tc.tile_pool(name="pc", bufs=PC_BUFS) as pcpool,
        ):
            # all psum flows through one pool of [P, FUSE, 512] bf16 tiles
            # (4 banks each); the f32 projections borrow tiles via bitcast
            def psb_f32(name):
                t = psbpool.tile(
                    [P, FUSE, 512], BF16, tag="psb", name=name
                )
                return t.bitcast(F32).rearrange("p i x -> p (i x)")

            inA_sb = cpool.tile([P, CA], BF16, tag="inA")
            inB_sb = cpool.tile([P, CB], BF16, tag="inB")
            x1T_sb = inA_sb[:, 0 : KO * P].rearrange("p (o i) -> p o i", o=KO)
            eye_sb = inA_sb[:, KO * P : KO * P + P]
            w1_sb = inA_sb[:, KO * P + P :].rearrange(
                "p (o d) -> p o d", o=KO
            )
            x2T_sb = inB_sb[:, 0 : KO * P].rearrange("p (o i) -> p o i", o=KO)
            w2_sb = inB_sb[:, KO * P :].rearrange("p (o d) -> p o d", o=KO)
            bias_sb = cpool.tile([1, DO], BF16, tag="bias")
            ones_bf = cpool.tile([2, P], BF16, tag="ones_bf")
            idx_sb = cpool.tile([P, max(WB_CHUNKS)], I32, tag="ctxidx")
            p1T_sb = cpool.tile([P, DC, NI], BF16, tag="p1T")
            p2bf_sb = cpool.tile([P, DO], BF16, tag="p2bf")
            warm_sb = cpool.tile([1, 2], F16, tag="warm")
            # whole output shard stays resident: [j, i, d] f16
            ob = cpool.tile([P, NI, DO], F16, tag="ob")

            nc.vector.memset(ones_bf[:], 1.0)
            nc.vector.memset(idx_sb[:], 0)
            # trigger the Act engine's activation-table load during the
            # input-DMA window instead of before the first psum copy
            nc.scalar.mul(warm_sb[:], ones_bf[0:1, 0:2], 1.0)

            if WARM_MMS:
                warm_ps = psb_f32("warm_ps")
                for wi in range(WARM_MMS):
                    nc.tensor.matmul(
                        warm_ps[:, 0:P],
                        ones_bf[0:2, :],
                        ones_bf[0:2, :],
                        start=True,
                        stop=True,
                    )

            # --- input DMAs (issue order = SP queue order).  w1 first: the
            # p1T chain (proj + copy + transposes) is longer than p2's.
            # bias rides the (otherwise idle) Act queue.  The last w2 piece
            # is small so its completion (+900 ns sem prop) lands early. ---
            nc.scalar.dma_start(out=bias_sb[:], in_=bias_d.ap())
            cA1 = KO * P + P + 3 * DO  # x1T + eye + w1 chunks 0-2
            nc.sync.dma_start(out=inA_sb[:, 0:cA1], in_=inA_d.ap()[:, 0:cA1])
            nc.sync.dma_start(out=inA_sb[:, cA1:CA], in_=inA_d.ap()[:, cA1:CA])
            cB1 = KO * P + 3 * DO  # x2T + w2 chunks 0-2
            cB2 = cB1 + 2 * DO  # w2 chunks 3-4
            nc.sync.dma_start(out=inB_sb[:, 0:cB1], in_=inB_d.ap()[:, 0:cB1])
            nc.sync.dma_start(out=inB_sb[:, cB1:cB2], in_=inB_d.ap()[:, cB1:cB2])
            nc.sync.dma_start(out=inB_sb[:, cB2:CB], in_=inB_d.ap()[:, cB2:CB])

            if "proj" not in SKIP:
                # --- p1T[d, i] = sum_k W1[k, d] x1[i, k] + bias[d], computed
                # directly transposed: lhsT = W1 128x128 blocks (slices of
                # w1r), rhs = x1T chunks.  One accumulation group, disjoint
                # 128-col ranges per d-chunk. ---
                p1T_ps = psb_f32("p1T_ps")
                for dc in range(DC):
                    nc.tensor.matmul(
                        p1T_ps[:, dc * P : (dc + 1) * P],
                        bias_sb[0:1, dc * P : (dc + 1) * P],
                        ones_bf[0:1, :],
                        start=(dc == 0),
                        stop=False,
                    )
                for o in range(KO):
                    for dc in range(DC):
                        nc.tensor.matmul(
                            p1T_ps[:, dc * P : (dc + 1) * P],
                            w1_sb[:, o, dc * P : (dc + 1) * P],
                            x1T_sb[:, o, :],
                            start=False,
                            stop=(o == KO - 1 and dc == DC - 1),
                        )
                nc.scalar.mul(
                    p1T_sb.rearrange("p c i -> p (c i)"),
                    p1T_ps[:, 0 : DC * P],
                    1.0,
                )

                # --- p2 = x2 @ W2h (f32 psum -> bf16 SBUF via Act) ---
                p2_ps = psb_f32("p2_ps")
                for o in range(KO):
                    nc.tensor.matmul(
                        p2_ps[:, 0:DO],
                        x2T_sb[:, o, :],
                        w2_sb[:, o, :],
                        start=(o == 0),
                        stop=(o == KO - 1),
                    )
                nc.scalar.mul(p2bf_sb[:], p2_ps[:, 0:DO], 1.0)

            # --- main loop: 32 four-row tiles into the resident ob ---
            p2_b = p2bf_sb[:, None, :].to_broadcast((P, FUSE, DO))
            wb_done = 0
            wb_iter = iter(WB_CHUNKS)
            wb_next = next(wb_iter)
            for t in range(NI // FUSE):
                i0 = FUSE * t
                ty = PATTERN[t]
                # [P, 4, 512] bf16 = 2 psum banks; rows at 1 KiB offsets so
                # each pair of rows sits in its own zero region
                ps = psbpool.tile([P, FUSE, 512], BF16, tag="psb", name=f"ps{t}")
                if "mm1" not in SKIP:
                    n = 0
                    for m in range(FUSE):
                        i = i0 + m
                        for c in range(DC):
                            nc.tensor.matmul(
                                ps[:, m, c * P : (c + 1) * P],
                                p1T_sb[:, c, i : i + 1].to_broadcast((P, P)),
                                eye_sb[:],
                                is_transpose=True,
                                start=(n % 6 == 0),
                                stop=(n % 6 == 5),
                            )
                            n += 1
                ps_v = ps[:, :, 0:DO]
                if "add" not in SKIP:
                    if ty == "D":
                        nc.vector.tensor_add(
                            out=ob[:, i0 : i0 + FUSE, :], in0=ps_v, in1=p2_b
                        )
                    else:  # P: Act moves psum pairs as f32, Pool adds p2
                        pc = pcpool.tile(
                            [P, FUSE, DO], BF16, tag="pc", name=f"pc{t}"
                        )
                        nc.scalar.mul(
                            pc[:].bitcast(F32),
                            ps.bitcast(F32)[:, :, 0:DOH],
                            1.0,
                        )
                        nc.gpsimd.tensor_add(
                            out=ob[:, i0 : i0 + FUSE, :], in0=pc[:], in1=p2_b
                        )
                if "wb" not in SKIP and i0 + FUSE == wb_done + wb_next:
                    b0 = wb_done
                    nb = wb_next
                    out_ap = (
                        out_d.ap()[b0 : b0 + nb]
                        .rearrange("i (j o) d -> i j o d", o=1)
                    )
                    in_ap = (
                        ob[:, b0 : b0 + nb, :]
                        .bitcast(F32)
                        .rearrange("j (o i) d -> j o i d", o=1)
                    )
                    nc.gpsimd.kv_writeback(
                        out_ap=out_ap,
                        in_ap=in_ap,
                        ctx_idxs_ap=idx_sb[:, 0:nb],
                        wraparound=False,
                        prepare_only=False,
                    )
                    wb_done += nb
                    wb_next = next(wb_iter, 0)

    nc.compile()
    return nc


def _get_module():
    key = (
        PSB_BUFS,
        PC_BUFS,
        WARM_MMS,
        PATTERN,
        WB_CHUNKS,
        tuple(sorted(SKIP)),
    )
    if key not in _cache:
        _cache[key] = _build_module()
    return _cache[key]


def _bf16(x):
    import ml_dtypes

    return np.asarray(x, dtype=np.float32).astype(ml_dtypes.bfloat16)


def _prep_xT(x):
    """[128, 768] -> [128, KO, 128] transposed chunk layout (bf16)."""
    return np.ascontiguousarray(x.T.reshape(KO, P, P).transpose(1, 0, 2))


def _make_in_maps(input1, input2, W, b):
    import ml_dtypes

    input1 = np.asarray(input1, dtype=np.float32)
    input2 = np.asarray(input2, dtype=np.float32)
    W = np.asarray(W, dtype=np.float32)
    b = np.asarray(b, dtype=np.float32)

    eye = np.eye(P, dtype=ml_dtypes.bfloat16)
    in_maps = []
    for c in range(NCORES):
        bb, h = divmod(c, 2)
        W1 = _bf16(W[:768, h * DO : (h + 1) * DO])
        W2 = _bf16(W[768:, h * DO : (h + 1) * DO])
        w1r = W1.reshape(KO, P, DO).transpose(1, 0, 2).reshape(P, KO * DO)
        w2r = W2.reshape(KO, P, DO).transpose(1, 0, 2).reshape(P, KO * DO)
        x1T = _prep_xT(_bf16(input1[bb])).reshape(P, KO * P)
        x2T = _prep_xT(_bf16(input2[bb])).reshape(P, KO * P)
        in_maps.append(
            {
                "inA": np.ascontiguousarray(
                    np.concatenate([x1T, eye, w1r], axis=1)
                ),
                "inB": np.ascontiguousarray(
                    np.concatenate([x2T, w2r], axis=1)
                ),
                "biasr": _bf16(b[h * DO : (h + 1) * DO]).reshape(1, DO),
            }
        )
    return in_maps


def kernel(input1, input2, W, b):
    from concourse import bass_utils

    suppress_trace = False
    if os.environ.get("BASS_TRACE"):
        try:
            from antenv.axon_hooks import get_axon_ntff_profile_hook  # noqa: F401
        except Exception:
            suppress_trace = True
    prev = os.environ.get("BASS_NEVER_TRACE")
    if suppress_trace:
        os.environ["BASS_NEVER_TRACE"] = "1"
    try:
        nc = _get_module()
        in_maps = _make_in_maps(input1, input2, W, b)
        res = bass_utils.run_bass_kernel_spmd(
            nc, in_maps, core_ids=list(range(NCORES))
        )
    finally:
        if suppress_trace:
            if prev is None:
                os.environ.pop("BASS_NEVER_TRACE", None)
            else:
                os.environ["BASS_NEVER_TRACE"] = prev
    out = np.empty((4, NI, NJ, 2 * DO), dtype=np.float32)
    for c in range(NCORES):
        bb, h = divmod(c, 2)
        raw = np.ascontiguousarray(np.asarray(res.results[c]["out"]))
        out[bb, :, :, h * DO : (h + 1) * DO] = raw.view(np.float16).astype(
            np.float32
        )
    return out
